# revision 1
# baseline (speedup 1.0000x reference)
"""BiLSTM-CRF token-mean NLL loss on 8 Trainium2 NeuronCores.

Sharding: 8 cores = 2 LSTM directions x 4 batch-quarters (B_l=16).
Each core runs: input projection (x @ W_ih^T + b), the 512-step LSTM
recurrence for its direction, and its direction's half of the emission
projection. Host merges the two emission halves per batch-quarter and
computes the (tiny) CRF forward algorithm + gold-path score reduction.

Device layouts (per core):
  xT      [768, 8192] bf16   col p = l*16+b, l = processing step (bwd cores
                             get time-reversed x so the device program is SPMD)
  wih_t   [128, 6*16*128]    stationary tiles (kc, m) of W_ih^T, g-block rows x2
  whh_t   [128, 4*16*128]    stationary tiles (k, m) of W_hh^T, g-block rows x2
  bias    [128, 16] fp32     per-gate-tile bias (g-block x2)
  wo_t    [128, 4*9] bf16    stationary tiles of w_out (this dir's 512 hid cols)
  bias_o  [9, 1] fp32        b_out on fwd cores, 0 on bwd cores
  out: emisT [9, 8192] fp32  emission partial, col p = l*16+b (processing order)
"""

import numpy as np
import ml_dtypes

B, S, EMB = 64, 512, 768
HID = 512
NTAG = 9
BL = 16            # batch per core
NPOS = S * BL      # positions per core
KC_E = EMB // 128  # 6 k-chunks for projection
KC_H = HID // 128  # 4 k-chunks for recurrence
MT = 16            # gate tiles (4*HID/128)
UNROLL = 8

_CACHED = {}


def _build_neff1():
    import concourse.bass as bass
    import concourse.bacc as bacc
    import concourse.mybir as mybir
    import concourse.tile as tile
    from concourse.bass import ds

    f32 = mybir.dt.float32
    bf16 = mybir.dt.bfloat16

    nc = bacc.Bacc("TRN2", target_bir_lowering=False, debug=False)

    xT = nc.dram_tensor("xT", [EMB, NPOS], bf16, kind="ExternalInput")
    wih = nc.dram_tensor("wih", [128, KC_E * MT * 128], bf16, kind="ExternalInput")
    whh = nc.dram_tensor("whh", [128, KC_H * MT * 128], bf16, kind="ExternalInput")
    bias = nc.dram_tensor("bias", [128, MT], f32, kind="ExternalInput")
    wo = nc.dram_tensor("wo", [128, KC_H * NTAG], bf16, kind="ExternalInput")
    bias_o = nc.dram_tensor("bias_o", [NTAG, 1], f32, kind="ExternalInput")
    emisT = nc.dram_tensor("emisT", [NTAG, NPOS], f32, kind="ExternalOutput")

    xpT = nc.dram_tensor("xpT", [128, S * MT * BL], f32)      # internal
    h_all = nc.dram_tensor("h_all", [128, S * KC_H * BL], bf16)  # internal

    sig = mybir.ActivationFunctionType.Sigmoid
    mult = mybir.AluOpType.mult
    add = mybir.AluOpType.add
    subtract = mybir.AluOpType.subtract

    with tile.TileContext(nc) as tc:
        with (
            tc.tile_pool(name="wpool", bufs=1) as wpool,
            tc.tile_pool(name="xpool", bufs=3) as xpool,
            tc.tile_pool(name="gpool", bufs=3) as gpool,
            tc.tile_pool(name="opool", bufs=3) as opool,
            tc.tile_pool(name="pp", bufs=2, space="PSUM") as pp,
            tc.tile_pool(name="pp9", bufs=2, space="PSUM") as pp9,
        ):
            # --- resident weights ---
            wih_sb = wpool.tile([128, KC_E * MT * 128], bf16, tag="wih")
            whh_sb = wpool.tile([128, KC_H * MT * 128], bf16, tag="whh")
            bias_sb = wpool.tile([128, MT], f32, tag="bias")
            wo_sb = wpool.tile([128, KC_H * NTAG], bf16, tag="wo")
            bias_o_sb = wpool.tile([NTAG, 1], f32, tag="biaso")
            nc.sync.dma_start(out=wih_sb[:], in_=wih[:])
            nc.sync.dma_start(out=whh_sb[:], in_=whh[:])
            nc.sync.dma_start(out=bias_sb[:], in_=bias[:])
            nc.sync.dma_start(out=wo_sb[:], in_=wo[:])
            nc.sync.dma_start(out=bias_o_sb[:], in_=bias_o[:])

            # --- phase 1: input projection -> xpT ---
            # out tile (m, pc): xp.T[m-block, 512 positions]
            for pc in range(MT):
                xs6 = xpool.tile([128, KC_E * 512], bf16, tag="xs6")
                for kc in range(KC_E):
                    nc.sync.dma_start(
                        out=xs6[:, kc * 512:(kc + 1) * 512],
                        in_=xT[kc * 128:(kc + 1) * 128, pc * 512:(pc + 1) * 512],
                    )
                for m in range(MT):
                    ps = pp.tile([128, 512], f32, tag="ppj")
                    for kc in range(KC_E):
                        nc.tensor.matmul(
                            ps[:],
                            wih_sb[:, (kc * MT + m) * 128:(kc * MT + m) * 128 + 128],
                            xs6[:, kc * 512:(kc + 1) * 512],
                            start=(kc == 0),
                            stop=(kc == KC_E - 1),
                        )
                    xo = opool.tile([128, 512], f32, tag="xo")
                    nc.vector.tensor_scalar_add(xo[:], ps[:], bias_sb[:, m:m + 1])
                    # scatter to xpT: col (pc*32+ss)*256 + m*16 + b
                    nc.sync.dma_start(
                        out=xpT[:].rearrange("p (s r) -> p s r", r=MT * BL)[
                            :, pc * 32:(pc + 1) * 32, m * BL:(m + 1) * BL],
                        in_=xo[:].rearrange("p (ss b) -> p ss b", b=BL),
                    )

            # --- phase 2: recurrence ---
            h_sb = nc.alloc_sbuf_tensor("h_state", [128, 2 * KC_H * BL], bf16).ap()
            c_sb = nc.alloc_sbuf_tensor("c_state", [128, 2 * KC_H * BL], f32).ap()
            nc.vector.memset(h_sb[:], 0.0)
            nc.vector.memset(c_sb[:], 0.0)

            CW = MT * BL  # 256 cols of pre-activations per step

            HB = KC_H * BL  # 64: one gate-block of columns

            def rearr2(ap_full, off):
                # columns {off : off+64} u {off+128 : off+192} as a [128, 2, 64] AP
                return ap_full.rearrange("p (blk c) -> p blk c", c=2 * HB)[
                    :, :, off:off + HB]

            def step_body(iv, pi):
                po = 1 - pi
                xs = xpool.tile([128, CW], f32, tag="xs")
                (nc.sync if pi == 0 else nc.gpsimd).dma_start(
                    out=xs[:], in_=xpT[:, ds(iv * CW, CW)])
                ps = pp.tile([128, CW], f32, tag="prec")
                G = gpool.tile([128, CW], f32, tag="G")

                def mm_group(ms):
                    for m in ms:
                        for k in range(KC_H):
                            nc.tensor.matmul(
                                ps[:, m * BL:(m + 1) * BL],
                                whh_sb[:, (k * MT + m) * 128:(k * MT + m) * 128 + 128],
                                h_sb[:, pi * HB + k * BL: pi * HB + (k + 1) * BL],
                                start=(k == 0),
                                stop=(k == KC_H - 1),
                            )

                # Group A = i,f,g gate tiles: their whole nonlinearity + c-update
                # chain hides under group B's (o-gate) matmuls.
                mm_group([0, 1, 2, 3, 4, 5, 6, 7, 8, 9, 10, 11])
                ga = G[:, 0:12 * BL]
                nc.vector.tensor_tensor(
                    out=ga, in0=ps[:, 0:12 * BL], in1=xs[:, 0:12 * BL], op=add)
                nc.scalar.activation(ga, ga, sig)
                # tanh(g) = 2*sigmoid(2g) - 1 ; the 2g pre-scale is folded into
                # the g-rows of whh/wih/bias on the host.
                gs = G[:, 8 * BL:12 * BL]
                nc.vector.tensor_scalar(gs, gs, 2.0, -1.0, mult, add)
                t1 = gpool.tile([128, HB], f32, tag="t1")
                nc.vector.tensor_tensor(out=t1[:], in0=G[:, 0:4 * BL], in1=gs, op=mult)
                c_new = c_sb[:, po * HB:(po + 1) * HB]
                c_old = c_sb[:, pi * HB:(pi + 1) * HB]
                nc.vector.tensor_tensor(out=c_new, in0=G[:, 4 * BL:8 * BL], in1=c_old, op=mult)
                nc.vector.tensor_tensor(out=c_new, in0=c_new, in1=t1[:], op=add)
                tc_t = gpool.tile([128, HB], f32, tag="tc")
                nc.scalar.activation(tc_t[:], c_new, mybir.ActivationFunctionType.Tanh)

                mm_group([12, 13, 14, 15])
                go = G[:, 12 * BL:16 * BL]
                nc.vector.tensor_tensor(
                    out=go, in0=ps[:, 12 * BL:16 * BL], in1=xs[:, 12 * BL:16 * BL], op=add)
                nc.scalar.activation(go, go, sig)
                h_new = h_sb[:, po * HB:(po + 1) * HB]
                nc.vector.tensor_tensor(out=h_new, in0=go, in1=tc_t[:], op=mult)
                (nc.gpsimd if pi == 0 else nc.sync).dma_start(
                    out=h_all[:, ds(iv * HB, HB)], in_=h_new)

            def unrolled(iv0, unroll):
                for i in range(unroll):
                    step_body(iv0 + i, i % 2)

            tc.For_i_unrolled_general(
                0, S, 1, unrolled, max_unroll=UNROLL,
                hint_engines=(mybir.EngineType.PE, mybir.EngineType.DVE,
                              mybir.EngineType.Activation, mybir.EngineType.SP),
            )

            # --- phase 3: emissions ---
            for pc in range(MT):
                hs = xpool.tile([128, 32 * KC_H * BL], bf16, tag="hs")
                nc.sync.dma_start(
                    out=hs[:], in_=h_all[:, pc * 32 * KC_H * BL:(pc + 1) * 32 * KC_H * BL]
                )
                ps9 = pp9.tile([NTAG, 512], f32, tag="ps9")
                hsr = hs[:].rearrange("p (ss k b) -> p ss k b", k=KC_H, b=BL)
                for kc in range(KC_H):
                    nc.tensor.matmul(
                        ps9[:],
                        wo_sb[:, kc * NTAG:(kc + 1) * NTAG],
                        hsr[:, :, kc, :],
                        start=(kc == 0),
                        stop=(kc == KC_H - 1),
                    )
                eo = opool.tile([NTAG, 512], f32, tag="eo")
                nc.vector.tensor_scalar_add(eo[:], ps9[:], bias_o_sb[:, 0:1])
                nc.sync.dma_start(out=emisT[:, pc * 512:(pc + 1) * 512], in_=eo[:])

    nc.compile()
    return nc


def _prep_core_inputs(x, w_ih, w_hh, b_all, w_out, b_out, D, q):
    """Build the input dict for core (direction D, batch-quarter q)."""
    bf16 = ml_dtypes.bfloat16
    bs = slice(BL * q, BL * q + BL)
    xs = x[bs]                       # [16, S, EMB]
    if D == 1:
        xs = xs[:, ::-1, :]          # processing order = reversed time
    # xT[e, l*16+b] = xs[b, l, e]
    xT = np.ascontiguousarray(xs.transpose(2, 1, 0).reshape(EMB, NPOS)).astype(bf16)

    gscale = np.ones((4 * HID,), np.float32)
    gscale[2 * HID:3 * HID] = 2.0    # pytorch gate order i,f,g,o -> g block

    wihs = (w_ih * gscale[:, None]).astype(np.float32)   # [2048, 768]
    whhs = (w_hh * gscale[:, None]).astype(np.float32)   # [2048, 512]
    bs_ = (b_all * gscale).astype(np.float32)            # [2048]

    # wih tiles: [kr, (kc*MT+m)*128+mc] = wihs[m*128+mc, kc*128+kr]
    wt = wihs.reshape(MT, 128, KC_E, 128).transpose(3, 2, 0, 1)   # [kr, kc, m, mc]
    wih_t = np.ascontiguousarray(wt.reshape(128, KC_E * MT * 128)).astype(bf16)
    ht = whhs.reshape(MT, 128, KC_H, 128).transpose(3, 2, 0, 1)
    whh_t = np.ascontiguousarray(ht.reshape(128, KC_H * MT * 128)).astype(bf16)
    bias_t = np.ascontiguousarray(bs_.reshape(MT, 128).T).astype(np.float32)

    # wo tiles: [kr, kc*9+t] = w_out[t, D*512 + kc*128 + kr]
    wo_half = w_out[:, D * HID:(D + 1) * HID]            # [9, 512]
    wo_t = np.ascontiguousarray(
        wo_half.reshape(NTAG, KC_H, 128).transpose(2, 1, 0).reshape(128, KC_H * NTAG)
    ).astype(bf16)
    bias_o = (b_out.reshape(NTAG, 1) if D == 0 else np.zeros((NTAG, 1))).astype(np.float32)

    return {
        "xT": np.asarray(xT), "wih": wih_t, "whh": whh_t, "bias": bias_t,
        "wo": wo_t, "bias_o": bias_o,
    }


def _crf_loss_host(emis, tags, mask, start_trans, end_trans, trans):
    """emis [S, B, T] fp32 (time-major), tags [S, B], mask [S, B]. Exact numpy CRF."""
    Sq, Bq, T = emis.shape
    bidx = np.arange(Bq)
    m = mask.astype(np.float64)
    e = emis.astype(np.float64)
    tr = trans.astype(np.float64)
    num = start_trans.astype(np.float64)[tags[0]] + e[0, bidx, tags[0]]
    trans_steps = tr[tags[:-1], tags[1:]]
    emit_steps = np.take_along_axis(e[1:], tags[1:, :, None], axis=2)[..., 0]
    num = num + ((trans_steps + emit_steps) * m[1:]).sum(0)
    last_idx = m.sum(0).astype(np.int64) - 1
    num = num + end_trans.astype(np.float64)[tags[last_idx, bidx]]

    alpha = start_trans.astype(np.float64) + e[0]        # [B, T]
    for t in range(1, Sq):
        x = alpha[:, :, None] + tr[None] + e[t][:, None, :]
        mx = x.max(1)
        nxt = mx + np.log(np.exp(x - mx[:, None, :]).sum(1))
        alpha = np.where(m[t][:, None] > 0, nxt, alpha)
    z = alpha + end_trans.astype(np.float64)
    mz = z.max(1)
    den = mz + np.log(np.exp(z - mz[:, None]).sum(1))
    llh = num - den
    return -(llh.sum() / m.sum())


def kernel(x, mask, target_tag, w_ih_f, w_hh_f, b_f, w_ih_b, w_hh_b, b_b,
           w_out, b_out, start_trans, end_trans, trans):
    from concourse.bass_utils import run_bass_kernel_spmd

    x = np.asarray(x, np.float32)
    mask = np.asarray(mask)
    target_tag = np.asarray(target_tag)
    w_out = np.asarray(w_out, np.float32)
    b_out = np.asarray(b_out, np.float32)

    if "nc" not in _CACHED:
        _CACHED["nc"] = _build_neff1()
    nc = _CACHED["nc"]

    in_maps = []
    for core in range(8):
        D, q = core // 4, core % 4
        w_ih = np.asarray(w_ih_f if D == 0 else w_ih_b, np.float32)
        w_hh = np.asarray(w_hh_f if D == 0 else w_hh_b, np.float32)
        b_all = np.asarray(b_f if D == 0 else b_b, np.float32)
        in_maps.append(_prep_core_inputs(x, w_ih, w_hh, b_all, w_out, b_out, D, q))

    res = run_bass_kernel_spmd(nc, in_maps, core_ids=list(range(8)))

    # merge emissions: emis[s, b, t]
    emis = np.zeros((S, B, NTAG), np.float32)
    for core in range(8):
        D, q = core // 4, core % 4
        eT = res.results[core]["emisT"]                 # [9, S*16] processing order
        e = eT.reshape(NTAG, S, BL).transpose(1, 2, 0)  # [S(proc), 16, 9]
        if D == 1:
            e = e[::-1]
        emis[:, BL * q:BL * q + BL, :] += e

    loss = _crf_loss_host(
        emis, np.asarray(target_tag).T, np.asarray(mask).T.astype(np.float32),
        np.asarray(start_trans, np.float32), np.asarray(end_trans, np.float32),
        np.asarray(trans, np.float32),
    )
    return np.float32(loss)

